# revision 17
# baseline (speedup 1.0000x reference)
"""Two-branch 2-layer GCN (EncoderGCN2) on 8 trn2 NeuronCores.

Strategy (graph/data parallel per the sharding hint):
  - Destination-node sharding: each of the 8 cores owns 1/8 of the
    destination nodes of BOTH graphs (x and y).
  - GCN normalization factorized: with dis = deg^-1/2,
        out = diag(dis) * (A+I) * diag(dis) * (X W) + b
    so each core builds the dis-scaled feature table H0 = diag(dis) X W1
    (replicated; dis folded into X on the host), fetches rows h0[src]
    for its edges with the batched dma_gather custom instruction
    (int16 indices over table halves, rotated across the 4 SWDGE
    queues so the descriptor drain runs 4 wide), scatter-adds per
    128-destination block with a one-hot selection matrix on the
    TensorEngine (PSUM accumulation), and applies the dis[dst] scale
    + bias (+ReLU) in the block epilogue.
  - Self-loop terms are NOT gathered: each core's H0 table is written
    ROTATED so its own destination slice sits at row 0 (the rotation is
    pure host-side data: XT column order + rebased gather indices, so
    the SPMD instruction stream stays identical across cores). Each
    block's own rows are then contiguous at row 128*b and arrive via
    one sequential DMA per 8 blocks, entering the PSUM accumulation as
    an extra identity-selection tile.
  - The layer-1 epilogue directly computes H2pre = diag(dis) relu(out1) W2
    for the core's destination slice; an AllGather shares the full
    H2pre table with every core for layer 2's cross-partition gathers
    (the "halo exchange").
  - Phases are interleaved x/y so the AllGathers hide under the other
    graph's compute.

Tables / matmuls run in bf16 (fp32 PSUM accumulation, fp32 epilogues).
"""

import numpy as np
import ml_dtypes

import concourse.bass as bass
import concourse.bacc as bacc
import concourse.mybir as mybir
import concourse.tile as tile
from concourse.bass_utils import run_bass_kernel_spmd
from concourse.masks import make_identity

f32 = mybir.dt.float32
bf16 = mybir.dt.bfloat16
i16 = mybir.dt.int16

N = 50000
E = 800000
IN = 128
HID = 128
OUT = 64
NC = 8
SH = N // NC               # 6250 destination rows per core
NBLK = (SH + 127) // 128   # 49 blocks
SHP = NBLK * 128           # 6272 padded rows per core
NPAD = ((N + 127) // 128) * 128   # 50048
NTA = NPAD // 128          # 391 stage-A tiles
TBL2 = NC * SHP            # 50176 rows of the layer-2 table
HALF1 = NPAD // 2          # 25024 (H0 half)
HALF2 = TBL2 // 2          # 25088 (H2 half)

GCAP = 16                  # max tiles per dma_gather (2048 idxs)
NQ = 4                     # SWDGE queues (ucode max)
SK = 8                     # S_T tiles per DVE build op
ACHUNK = 32                # stage-A tiles per XT load
SELFGRP = 8                # blocks per sequential self-tile DMA
GB = 4                     # blocks per shared PSUM bank / epilogue batch


def _layer_meta(es_by_core, blk_by_core, off_by_core, half_split):
    """Per-layer tile schedule with edges split by table half.

    Tiles per block: 1 sequential self tile (identity selection over the
    block's own contiguous table rows) followed by gather tiles for the
    lo/hi table halves. The gather-tile ordinals index the idx table;
    sched positions index dstl and the matmul order.
    """
    ntl = np.zeros((NC, NBLK), np.int64)
    nth = np.zeros((NC, NBLK), np.int64)
    parts = []
    for c in range(NC):
        es, b, o = es_by_core[c], blk_by_core[c], off_by_core[c]
        lo = es < half_split
        parts.append((es, b, o, lo))
        for blk in range(NBLK):
            m = b == blk
            nlo = int((lo & m).sum())
            nhi = int((~lo & m).sum())
            ntl[c, blk] = (nlo + 127) // 128
            nth[c, blk] = (nhi + 127) // 128
    NTL = ntl.max(axis=0)
    NTH = nth.max(axis=0)
    TT = int(NBLK + NTL.sum() + NTH.sum())   # sched positions (incl. self)
    TG = int(NTL.sum() + NTH.sum())          # gather tiles

    # sched: per tile position (block, first, last, gather_ordinal|-1)
    # groups: (gather_ordinal_start, ntiles, half) — idx-table slices
    sched = []
    groups = []
    gbase_lo = np.zeros(NBLK, np.int64)   # sched position of first lo tile
    gbase_hi = np.zeros(NBLK, np.int64)
    obase_lo = np.zeros(NBLK, np.int64)   # gather ordinal of first lo tile
    obase_hi = np.zeros(NBLK, np.int64)
    t = 0
    g = 0
    for blk in range(NBLK):
        ngat = int(NTL[blk] + NTH[blk])
        sched.append((blk, True, ngat == 0, -1))         # self tile
        gbase_lo[blk] = t + 1
        gbase_hi[blk] = t + 1 + int(NTL[blk])
        obase_lo[blk] = g
        obase_hi[blk] = g + int(NTL[blk])
        for i in range(ngat):
            sched.append((blk, False, i == ngat - 1, g + i))
        for half, cnt in ((0, int(NTL[blk])), (1, int(NTH[blk]))):
            s = g if half == 0 else g + int(NTL[blk])
            while cnt > 0:
                n = min(cnt, GCAP)
                groups.append((s, n, half))
                s += n
                cnt -= n
        t += 1 + ngat
        g += ngat
    assert t == TT and g == TG

    idx16 = np.zeros((NC, TG * 128), np.int16)
    dstl = np.full((NC, TT * 128), 255.0, np.float32)
    # self tiles: dstl = slot index (identity selection)
    for blk in range(NBLK):
        base = int(np.where(
            [s[0] == blk and s[3] == -1 for s in sched])[0][0]) * 128
        dstl[:, base:base + 128] = np.arange(128, dtype=np.float32)
    for c in range(NC):
        es, b, o, lo = parts[c]
        for half, mask, gos, sps, halfoff in (
                (0, lo, obase_lo, gbase_lo, 0),
                (1, ~lo, obase_hi, gbase_hi, half_split)):
            eh, bh, oh = es[mask], b[mask], o[mask]
            cnt = np.bincount(bh, minlength=NBLK)
            starts = np.zeros(NBLK, np.int64)
            starts[1:] = np.cumsum(cnt)[:-1]
            pos = np.arange(len(eh)) - starts[bh]
            idx16[c, gos[bh] * 128 + pos] = (eh - halfoff).astype(np.int16)
            dstl[c, sps[bh] * 128 + pos] = oh
    # dma_gather wrapped layout: flat j -> [16k + j%16, j//16], replicated
    wrap = idx16.reshape(NC, TG * 8, 16).transpose(0, 2, 1)       # [NC,16,TG*8]
    idx16w = np.tile(wrap, (1, 8, 1)).copy()                      # [NC,128,TG*8]
    dstlw = dstl.reshape(NC, TT, 128).transpose(0, 2, 1)
    dstlw = dstlw.astype(ml_dtypes.bfloat16).copy()               # [NC,128,TT]
    return dict(TT=TT, TG=TG, sched=sched, groups=groups, idx=idx16w,
                dstl=dstlw)


def _graph_meta(edge_index):
    src = np.asarray(edge_index[0]).astype(np.int64)
    dst = np.asarray(edge_index[1]).astype(np.int64)
    deg = (np.bincount(dst, minlength=N) + 1).astype(np.float32)
    dis = (1.0 / np.sqrt(deg)).astype(np.float32)

    core = dst // SH
    esA, esC, blks, offs = [], [], [], []
    disb = np.zeros((NC, 128, NBLK), np.float32)
    for c in range(NC):
        m = core == c
        es, ed = src[m], dst[m]
        l = ed - c * SH
        order = np.argsort(l, kind="stable")
        es, l = es[order], l[order]
        esA.append((es - c * SH) % N)     # per-core rotated H0 position
        esC.append((es // SH) * SHP + (es % SH))
        blks.append(l >> 7)
        offs.append(l & 127)
        lv = np.arange(SHP)
        gv = np.minimum(c * SH + lv, N - 1)
        disb[c] = np.where(lv < SH, dis[gv], 0.0).reshape(NBLK, 128).T

    l1 = _layer_meta(esA, blks, offs, HALF1)
    l2 = _layer_meta(esC, blks, offs, HALF2)
    return dict(l1=l1, l2=l2, disb=disb, dis=dis)


def _build(meta):
    DTT = bf16

    nc = bacc.Bacc("TRN2", target_bir_lowering=False, debug=False,
                   num_devices=NC, num_swdge_queues=NQ)
    gs = ("x", "y")
    XT, W1, W2, B1, B2 = {}, {}, {}, {}, {}
    IDX1, IDX2, DSTL1, DSTL2, DISB = {}, {}, {}, {}, {}
    H0, H2loc, H2all, OUTT = {}, {}, {}, {}
    for g in gs:
        T1, G1 = meta[g]["l1"]["TT"], meta[g]["l1"]["TG"]
        T2, G2 = meta[g]["l2"]["TT"], meta[g]["l2"]["TG"]
        XT[g] = nc.dram_tensor(f"xt_{g}", [128, NPAD], DTT, kind="ExternalInput")
        W1[g] = nc.dram_tensor(f"w1_{g}", [128, HID], DTT, kind="ExternalInput")
        W2[g] = nc.dram_tensor(f"w2_{g}", [128, OUT], DTT, kind="ExternalInput")
        B1[g] = nc.dram_tensor(f"b1_{g}", [128, HID], f32, kind="ExternalInput")
        B2[g] = nc.dram_tensor(f"b2_{g}", [128, OUT], f32, kind="ExternalInput")
        IDX1[g] = nc.dram_tensor(f"idx1_{g}", [128, G1 * 8], i16, kind="ExternalInput")
        IDX2[g] = nc.dram_tensor(f"idx2_{g}", [128, G2 * 8], i16, kind="ExternalInput")
        DSTL1[g] = nc.dram_tensor(f"dstl1_{g}", [128, T1], DTT, kind="ExternalInput")
        DSTL2[g] = nc.dram_tensor(f"dstl2_{g}", [128, T2], DTT, kind="ExternalInput")
        DISB[g] = nc.dram_tensor(f"disb_{g}", [128, NBLK], f32, kind="ExternalInput")
        H0[g] = nc.dram_tensor(f"h0_{g}", [NPAD, HID], DTT)
        H2loc[g] = nc.dram_tensor(f"h2loc_{g}", [SHP, 128], DTT)
        H2all[g] = nc.dram_tensor(f"h2all_{g}", [TBL2, 128], DTT)
        OUTT[g] = nc.dram_tensor(f"out_{g}", [SHP, OUT], f32, kind="ExternalOutput")
    IOTA = nc.dram_tensor("iota", [128, 128], DTT, kind="ExternalInput")

    qctr = [0]

    with tile.TileContext(nc) as tc:
        with (
            tc.tile_pool(name="meta", bufs=1) as mp,
            tc.tile_pool(name="xt", bufs=3) as xp,
            tc.tile_pool(name="h0s", bufs=3) as h0p,
            tc.tile_pool(name="gat", bufs=8) as gp,
            tc.tile_pool(name="slf", bufs=3) as sp,
            tc.tile_pool(name="st", bufs=3) as stp,
            tc.tile_pool(name="epi", bufs=8) as ep,
            tc.tile_pool(name="stage", bufs=2) as sgp,
            tc.tile_pool(name="psA", bufs=2, space="PSUM") as ppa,
            tc.tile_pool(name="psE", bufs=3, space="PSUM") as ppe,
            tc.tile_pool(name="psX", bufs=2, space="PSUM") as ppx,
        ):
            iota_sb = mp.tile([128, 128], DTT)
            nc.sync.dma_start(out=iota_sb[:], in_=IOTA[:, :])
            ident = mp.tile([128, 128], DTT)
            make_identity(nc, ident[:])
            # zero the gather pool once: slots skipped by the -1 padding
            # are read (x0) by the one-hot matmuls and must be finite
            for _ in range(8):
                warm = gp.tile([128, GCAP * 128], DTT, tag="gt")
                nc.vector.memset(warm[:], 0.0)
            w1_sb, w2_sb, b1_sb, b2_sb = {}, {}, {}, {}
            idx_sb, dstl_sb, disb_sb = {}, {}, {}
            for g in gs:
                T1, G1 = meta[g]["l1"]["TT"], meta[g]["l1"]["TG"]
                T2, G2 = meta[g]["l2"]["TT"], meta[g]["l2"]["TG"]
                w1_sb[g] = mp.tile([128, HID], DTT, tag=f"w1{g}", name=f"w1sb_{g}")
                nc.sync.dma_start(out=w1_sb[g][:], in_=W1[g][:, :])
                w2_sb[g] = mp.tile([128, OUT], DTT, tag=f"w2{g}", name=f"w2sb_{g}")
                nc.sync.dma_start(out=w2_sb[g][:], in_=W2[g][:, :])
                b1_sb[g] = mp.tile([128, HID], f32, tag=f"b1{g}", name=f"b1sb_{g}")
                nc.sync.dma_start(out=b1_sb[g][:], in_=B1[g][:, :])
                b2_sb[g] = mp.tile([128, OUT], f32, tag=f"b2{g}", name=f"b2sb_{g}")
                nc.sync.dma_start(out=b2_sb[g][:], in_=B2[g][:, :])
                idx_sb[g, 1] = mp.tile([128, G1 * 8], i16, tag=f"i1{g}", name=f"idx1sb_{g}")
                nc.sync.dma_start(out=idx_sb[g, 1][:], in_=IDX1[g][:, :])
                idx_sb[g, 2] = mp.tile([128, G2 * 8], i16, tag=f"i2{g}", name=f"idx2sb_{g}")
                nc.sync.dma_start(out=idx_sb[g, 2][:], in_=IDX2[g][:, :])
                dstl_sb[g, 1] = mp.tile([128, T1], DTT, tag=f"d1{g}", name=f"dstl1sb_{g}")
                nc.sync.dma_start(out=dstl_sb[g, 1][:], in_=DSTL1[g][:, :])
                dstl_sb[g, 2] = mp.tile([128, T2], DTT, tag=f"d2{g}", name=f"dstl2sb_{g}")
                nc.sync.dma_start(out=dstl_sb[g, 2][:], in_=DSTL2[g][:, :])
                disb_sb[g] = mp.tile([128, NBLK], f32, tag=f"db{g}", name=f"disbsb_{g}")
                nc.sync.dma_start(out=disb_sb[g][:], in_=DISB[g][:, :])

            def stage_A(g):
                # H0 = (diag(dis) X) W1 — dis folded into XT on the host.
                h0v = H0[g].ap().rearrange("(nb p) c -> nb p c", p=128)
                for t0 in range(0, NTA, ACHUNK):
                    csz = min(ACHUNK, NTA - t0)
                    xtc = xp.tile([128, csz * 128], DTT, tag="xtc")
                    nc.sync.dma_start(
                        out=xtc[:], in_=XT[g][:, t0 * 128:(t0 + csz) * 128])
                    h0s = h0p.tile([128, csz * 128], DTT, tag="h0s")
                    for s0 in range(0, csz, 4):
                        ssz = min(4, csz - s0)
                        ps = ppa.tile([128, ssz * 128], f32, space="PSUM", tag="psA")
                        for i in range(ssz):
                            nc.tensor.matmul(
                                out=ps[:, i * 128:(i + 1) * 128],
                                lhsT=xtc[:, (s0 + i) * 128:(s0 + i + 1) * 128],
                                rhs=w1_sb[g][:],
                                start=True, stop=True,
                            )
                        nc.vector.tensor_copy(
                            out=h0s[:, s0 * 128:(s0 + ssz) * 128],
                            in_=ps[:])
                    nc.sync.dma_start(
                        out=h0v[t0:t0 + csz].rearrange("nb p c -> p nb c"),
                        in_=h0s[:].rearrange("p (nb c) -> p nb c", c=128),
                    )

            def edge_stage(g, layer):
                m = meta[g]["l1" if layer == 1 else "l2"]
                TT, TG, sched, groups = m["TT"], m["TG"], m["sched"], m["groups"]
                D = HID if layer == 1 else OUT
                GW = 128                       # gathered row width (cols)
                if layer == 1:
                    halves = (H0[g][0:HALF1, :], H0[g][HALF1:2 * HALF1, :])
                    selfsrc = H0[g].ap()       # rotated: own rows at 128b
                else:
                    halves = (H2all[g][0:HALF2, :], H2all[g][HALF2:2 * HALF2, :])
                    selfsrc = H2loc[g].ap()    # own rows at 128b
                isb = idx_sb[g, layer]
                dsb = dstl_sb[g, layer]

                gt_of = [None] * TT            # sched position -> (tile, off)
                gidx = 0
                ps = None
                stb = None
                s0 = 0
                self_t = None
                pending = []                   # (block, psum tile) awaiting epilogue

                def flush(bb, nst, st_t):
                    lo = bb - nst + 1
                    if layer == 1:
                        dst = H2loc[g].ap().rearrange("(nb p) c -> nb p c", p=128)
                        w = 128
                    else:
                        dst = OUTT[g].ap().rearrange("(nb p) c -> nb p c", p=128)
                        w = OUT
                    nc.sync.dma_start(
                        out=dst[lo:lo + nst].rearrange("nb p c -> p nb c"),
                        in_=st_t[:, :nst * w].rearrange("p (nb c) -> p nb c", c=w),
                    )

                # map gather ordinal -> sched position (for tile handles)
                ord2pos = [None] * TG
                for j, (_, _, _, go) in enumerate(sched):
                    if go >= 0:
                        ord2pos[go] = j

                for j in range(TT):
                    b, first, last, go = sched[j]
                    if first:
                        # stage the self tiles for this 8-block group
                        if b % SELFGRP == 0:
                            nblks = min(SELFGRP, NBLK - b)
                            self_t = sp.tile([128, SELFGRP * 128], DTT, tag="slf")
                            r0 = b * 128
                            nc.sync.dma_start(
                                out=self_t[:, :nblks * 128]
                                    .rearrange("p (nb c) -> p nb c", c=128),
                                in_=selfsrc[r0:r0 + nblks * 128, :]
                                    .rearrange("(nb p) c -> p nb c", p=128),
                            )
                        gt_of[j] = (self_t, (b % SELFGRP) * 128)
                    if go >= 0 and gidx < len(groups) and groups[gidx][0] == go:
                        ts, ntl, half = groups[gidx]
                        gt = gp.tile([128, ntl * GW], DTT, tag="gt")
                        nc.gpsimd.dma_gather(
                            out_ap=gt[:].rearrange("p (t c) -> p t c", c=GW),
                            in_ap=halves[half],
                            idxs_ap=isb[:, ts * 8:(ts + ntl) * 8],
                            num_idxs=ntl * 128,
                            num_idxs_reg=ntl * 128,
                            elem_size=GW,
                            single_packet=False,
                            queue_num=qctr[0] % NQ,
                        )
                        qctr[0] += 1
                        for k in range(ntl):
                            gt_of[ord2pos[ts + k]] = (gt, k * GW)
                        gidx += 1
                    if j % SK == 0:
                        ssz = min(SK, TT - j)
                        stb = stp.tile([128, ssz * 128], DTT, tag="stb")
                        nc.vector.tensor_tensor(
                            out=stb[:].rearrange("p (t c) -> p t c", c=128),
                            in0=dsb[:, j:j + ssz]
                                .rearrange("p (t c) -> p t c", c=1)
                                .to_broadcast([128, ssz, 128]),
                            in1=iota_sb[:].rearrange("p (t c) -> p t c", t=1)
                                .to_broadcast([128, ssz, 128]),
                            op=mybir.AluOpType.is_equal,
                        )
                        s0 = j
                    if first:
                        if b % GB == 0:
                            pgrp = ppe.tile([128, GB * D], f32,
                                            space="PSUM", tag="psE")
                        ps = pgrp[:, (b % GB) * D:(b % GB + 1) * D]
                    gtile, goff = gt_of[j]
                    nc.tensor.matmul(
                        out=ps,
                        lhsT=stb[:, (j - s0) * 128:(j - s0 + 1) * 128],
                        rhs=gtile[:, goff:goff + D],
                        start=first, stop=last,
                    )
                    if last:
                        pending.append((b, ps))
                        if b % GB == GB - 1 or b == NBLK - 1:
                            # batched epilogue: keep the PE's edge-matmul
                            # stream unbroken for 8 blocks at a time
                            stage_t = sgp.tile(
                                [128, GB * (128 if layer == 1 else OUT)],
                                DTT if layer == 1 else f32,
                                tag=f"stage{layer}", name=f"staget_{g}{layer}")
                            for bb, pps in pending:
                                dcol = disb_sb[g][:, bb:bb + 1]
                                if layer == 1:
                                    z1 = ep.tile([128, HID], f32, tag="z1")
                                    nc.vector.tensor_scalar(
                                        out=z1[:], in0=pps, scalar1=dcol,
                                        scalar2=None, op0=mybir.AluOpType.mult)
                                    z2 = ep.tile([128, HID], f32, tag="z2")
                                    nc.vector.tensor_tensor(
                                        out=z2[:], in0=z1[:], in1=b1_sb[g][:],
                                        op=mybir.AluOpType.add)
                                    r = ep.tile([128, HID], DTT, tag="r")
                                    nc.scalar.activation(
                                        out=r[:], in_=z2[:],
                                        func=mybir.ActivationFunctionType.Relu)
                                    pst = ppx.tile([128, HID], DTT,
                                                   space="PSUM", tag="psX")
                                    nc.tensor.transpose(
                                        out=pst[:], in_=r[:], identity=ident[:])
                                    rt = ep.tile([128, HID], DTT, tag="rt")
                                    nc.vector.tensor_copy(out=rt[:], in_=pst[:])
                                    ph2 = ppx.tile([128, OUT], f32,
                                                   space="PSUM", tag="psX")
                                    nc.tensor.matmul(
                                        out=ph2[:], lhsT=rt[:], rhs=w2_sb[g][:],
                                        start=True, stop=True)
                                    nc.vector.tensor_scalar(
                                        out=stage_t[:, (bb % GB) * 128:
                                                    (bb % GB) * 128 + OUT],
                                        in0=ph2[:], scalar1=dcol,
                                        scalar2=None, op0=mybir.AluOpType.mult)
                                else:
                                    o1 = ep.tile([128, OUT], f32, tag="o1")
                                    nc.vector.tensor_scalar(
                                        out=o1[:], in0=pps, scalar1=dcol,
                                        scalar2=None, op0=mybir.AluOpType.mult)
                                    nc.vector.tensor_tensor(
                                        out=stage_t[:, (bb % GB) * OUT:
                                                    (bb % GB + 1) * OUT],
                                        in0=o1[:], in1=b2_sb[g][:],
                                        op=mybir.AluOpType.add)
                            flush(b, len(pending), stage_t)
                            pending = []

            def allgather(g):
                nc.gpsimd.collective_compute(
                    "AllGather",
                    mybir.AluOpType.bypass,
                    replica_groups=[list(range(NC))],
                    ins=[H2loc[g].ap().opt()],
                    outs=[H2all[g].ap().opt()],
                )

            stage_A("x")
            stage_A("y")
            edge_stage("x", 1)
            allgather("x")
            edge_stage("y", 1)
            allgather("y")
            edge_stage("x", 2)
            edge_stage("y", 2)

    nc.compile()
    return nc


def _in_maps(meta, inputs):
    np_t = ml_dtypes.bfloat16
    iota = np.broadcast_to(
        np.arange(128, dtype=np.float32), (128, 128)).astype(np_t).copy()
    weights = {
        "x": (inputs["W1x"], inputs["b1x"], inputs["W2x"], inputs["b2x"],
              inputs["x_data_matrix"]),
        "y": (inputs["W1y"], inputs["b1y"], inputs["W2y"], inputs["b2y"],
              inputs["y_data_matrix"]),
    }
    shared = {"iota": iota}
    xts = {}
    for g in ("x", "y"):
        w1, b1, w2, b2, xd = weights[g]
        dis = meta[g]["dis"]
        xts[g] = (np.asarray(xd, np.float32) * dis[:, None]).T.astype(np_t)
        shared[f"w1_{g}"] = np.asarray(w1, np.float32).astype(np_t)
        shared[f"w2_{g}"] = np.asarray(w2, np.float32).astype(np_t)
        shared[f"b1_{g}"] = np.broadcast_to(
            np.asarray(b1, np.float32), (128, HID)).copy()
        shared[f"b2_{g}"] = np.broadcast_to(
            np.asarray(b2, np.float32), (128, OUT)).copy()
    maps = []
    for c in range(NC):
        m = dict(shared)
        for g in ("x", "y"):
            mg = meta[g]
            # per-core rotated table order: column j holds node (c*SH+j)%N
            xt = np.zeros((128, NPAD), np_t)
            rot = (c * SH + np.arange(N)) % N
            xt[:, :N] = xts[g][:, rot]
            m[f"xt_{g}"] = xt
            m[f"idx1_{g}"] = mg["l1"]["idx"][c]
            m[f"idx2_{g}"] = mg["l2"]["idx"][c]
            m[f"dstl1_{g}"] = mg["l1"]["dstl"][c]
            m[f"dstl2_{g}"] = mg["l2"]["dstl"][c]
            m[f"disb_{g}"] = mg["disb"][c]
        maps.append(m)
    return maps


def run(inputs, trace=False):
    meta = {
        "x": _graph_meta(inputs["x_edge_index"]),
        "y": _graph_meta(inputs["y_edge_index"]),
    }
    nc = _build(meta)
    maps = _in_maps(meta, inputs)
    kwargs = {}
    if trace:
        kwargs = dict(trace=True, trace_cores=[0])
    res = run_bass_kernel_spmd(nc, maps, core_ids=list(range(NC)), **kwargs)
    outs = {}
    for g in ("x", "y"):
        full = np.empty((N, OUT), np.float32)
        for c in range(NC):
            full[c * SH:(c + 1) * SH] = res.results[c][f"out_{g}"][:SH]
        outs[g] = full
    return (outs["x"], outs["y"]), res


def kernel(**inputs):
    (ox, oy), _ = run(inputs)
    return ox, oy
